# revision 3
# baseline (speedup 1.0000x reference)
"""Trainium2 Bass kernel for nn_EncoderLayer (D=1024, H=16, S=2048, FF=4096), 8-core SPMD.

Strategy: head-parallel attention (2 heads/core), one 1MB AllToAll to switch to
sequence-parallel (256 positions/core) for the output projection, norms and FFN.
No all-reduce needed anywhere.
"""
import math

import numpy as np

import concourse.bass as bass
import concourse.mybir as mybir
import concourse.tile as tile
from concourse import bacc
from concourse.bass_utils import run_bass_kernel_spmd
from concourse.masks import make_identity

F32 = mybir.dt.float32
AF = mybir.ActivationFunctionType

D = 1024
H = 16
HD = 64
S = 2048
FF = 4096
EPS = 1e-3
NCORES = 8
SL = S // NCORES          # 256 sequence positions per core after A2A
HPC = H // NCORES         # 2 heads per core
KT = D // 128             # 8 k-tiles over the model dim
TT = S // 128             # 16 t-tiles over sequence
SCH = 512                 # free-dim chunk for score matmuls
NSCH = S // SCH           # 4 s-chunks
FFT = FF // 128           # 32 hidden tiles
UNBIAS = float(D) / float(D - 1)  # torch std uses ddof=1


def _ln(nc, pools, x_sb, z_sb, a2_sb, b2n_sb):
    """LayerNorm over free axis (1024) of x_sb [128, 1024] -> z_sb [128, 1024].

    Matches reference: (x - mu) / (std_ddof1 + eps) * a2 + b2.
    """
    stats = pools.tile([128, 2, 6], F32, tag="ln_stats")
    mv = pools.tile([128, 2], F32, tag="ln_mv")
    for g in range(2):
        nc.vector.bn_stats(out=stats[:, g, :], in_=x_sb[:, g * 512:(g + 1) * 512])
    nc.vector.bn_aggr(out=mv[:], in_=stats[:])
    sig = pools.tile([128, 1], F32, tag="ln_sig")
    # sigma = sqrt(var * N/(N-1)); then += eps; then reciprocal
    nc.scalar.activation(sig[:], mv[:, 1:2], AF.Sqrt, scale=UNBIAS)
    nc.vector.tensor_scalar_add(sig[:], sig[:], EPS)
    rec = pools.tile([128, 1], F32, tag="ln_rec")
    nc.vector.reciprocal(rec[:], sig[:])
    nc.vector.tensor_scalar(
        out=z_sb[:], in0=x_sb[:],
        scalar1=mv[:, 0:1], scalar2=rec[:],
        op0=mybir.AluOpType.subtract, op1=mybir.AluOpType.mult,
    )
    nc.vector.tensor_mul(out=z_sb[:], in0=z_sb[:], in1=a2_sb[:])
    nc.vector.tensor_add(out=z_sb[:], in0=z_sb[:], in1=b2n_sb[:])


def build(reps: int = 1):
    nc = bacc.Bacc("TRN2", target_bir_lowering=False, debug=False, num_devices=NCORES)

    # ---- DRAM parameters (per-core shards prepared on host) ----
    Qt = nc.declare_dram_parameter("Qt", [KT, 128, S], F32, isOutput=False)
    Kt = nc.declare_dram_parameter("Kt", [KT, 128, S], F32, isOutput=False)
    Vt = nc.declare_dram_parameter("Vt", [KT, 128, S], F32, isOutput=False)
    wqT = nc.declare_dram_parameter("wqT", [128, KT, 128], F32, isOutput=False)
    wkT = nc.declare_dram_parameter("wkT", [128, KT, 128], F32, isOutput=False)
    wvT = nc.declare_dram_parameter("wvT", [128, KT, 128], F32, isOutput=False)
    Wot = nc.declare_dram_parameter("Wot", [128, KT, D], F32, isOutput=False)
    W1t = nc.declare_dram_parameter("W1t", [FFT, 128, KT, 128], F32, isOutput=False)
    W2t = nc.declare_dram_parameter("W2t", [KT, 128, FFT, 128], F32, isOutput=False)
    b1t = nc.declare_dram_parameter("b1t", [128, FFT], F32, isOutput=False)
    b2t = nc.declare_dram_parameter("b2t", [128, KT], F32, isOutput=False)
    a2b = nc.declare_dram_parameter("a2b", [128, D], F32, isOutput=False)
    b2nb = nc.declare_dram_parameter("b2nb", [128, D], F32, isOutput=False)
    VsT = nc.declare_dram_parameter("VsT", [2, 128, D], F32, isOutput=False)
    outs = [
        nc.declare_dram_parameter(f"out{r}", [D, SL], F32, isOutput=True)
        for r in range(reps)
    ]

    with tile.TileContext(nc) as tc:
        with (
            tc.tile_pool(name="singles", bufs=1) as singles,
            tc.tile_pool(name="dram", bufs=2, space="DRAM") as dram,
        ):
            ident = singles.tile([128, 128], F32)
            make_identity(nc, ident[:])
            a2_sb = singles.tile([128, D], F32)
            b2n_sb = singles.tile([128, D], F32)
            b1_sb = singles.tile([128, FFT], F32)
            b2_sb = singles.tile([128, KT], F32)
            nc.sync.dma_start(a2_sb[:], a2b[:])
            nc.sync.dma_start(b2n_sb[:], b2nb[:])
            nc.sync.dma_start(b1_sb[:], b1t[:])
            nc.sync.dma_start(b2_sb[:], b2t[:])

            for r in range(reps):
                _body(nc, tc, singles, dram, ident, a2_sb, b2n_sb, b1_sb, b2_sb,
                      Qt, Kt, Vt, wqT, wkT, wvT, Wot, W1t, W2t, VsT, outs[r])

    nc.finalize()
    return nc


def _body(nc, tc, singles, dram, ident, a2_sb, b2n_sb, b1_sb, b2_sb,
          Qt, Kt, Vt, wqT, wkT, wvT, Wot, W1t, W2t, VsT, out):
    import contextlib
    with contextlib.ExitStack() as stack:
        attn = stack.enter_context(tc.tile_pool(name="attn", bufs=1))
        # ---------------- Phase A: projections ----------------
        vq_sb = attn.tile([128, S], F32)   # [2 heads * 64 d, s]
        vk_sb = attn.tile([128, S], F32)
        vvT_sb = attn.tile([128, TT, 2 * (HD + 1)], F32)  # [t_in, t_tile, (d+ones)*2]

        with (
            tc.tile_pool(name="projw", bufs=1) as projw,
            tc.tile_pool(name="projin", bufs=3) as projin,
            tc.tile_pool(name="projps", bufs=4, space="PSUM") as projps,
            tc.tile_pool(name="trps", bufs=2, space="PSUM") as trps,
        ):
            wq_sb = projw.tile([128, KT, 128], F32)
            wk_sb = projw.tile([128, KT, 128], F32)
            wv_sb = projw.tile([128, KT, 128], F32)
            nc.sync.dma_start(wq_sb[:], wqT[:])
            nc.sync.dma_start(wk_sb[:], wkT[:])
            nc.sync.dma_start(wv_sb[:], wvT[:])

            vv_sb = attn.tile([128, S], F32)
            for (src, wsb, dst) in ((Qt, wq_sb, vq_sb), (Kt, wk_sb, vk_sb), (Vt, wv_sb, vv_sb)):
                ps = [projps.tile([128, SCH], F32, tag="proj_ps", name=f"proj_ps{j}")
                      for j in range(NSCH)]
                for k in range(KT):
                    xin = projin.tile([128, S], F32, tag="proj_in")
                    nc.sync.dma_start(xin[:], src[k])
                    for j in range(NSCH):
                        nc.tensor.matmul(
                            ps[j][:], wsb[:, k, :], xin[:, j * SCH:(j + 1) * SCH],
                            start=(k == 0), stop=(k == KT - 1),
                        )
                for j in range(NSCH):
                    nc.vector.tensor_copy(dst[:, j * SCH:(j + 1) * SCH], ps[j][:])

            # transpose Vv [(h d), t] -> vvT [t, (d|1)*2] per t_tile, with ones col
            nc.gpsimd.memset(vvT_sb[:], 1.0)  # ones columns come for free
            for t in range(TT):
                pst = trps.tile([128, 128], F32, tag="tr_ps")
                nc.tensor.transpose(pst[:], vv_sb[:, t * 128:(t + 1) * 128], ident[:])
                nc.vector.tensor_copy(vvT_sb[:, t, 0:HD], pst[0:128, 0:HD].rearrange("p f -> p f"))
                nc.vector.tensor_copy(vvT_sb[:, t, HD + 1:2 * HD + 1], pst[:, HD:2 * HD])

        # ---------------- Phase B: attention per head ----------------
        send = dram.tile([NCORES, 128, SL], F32, tag="send")
        with (
            tc.tile_pool(name="esb", bufs=1) as esb,
            tc.tile_pool(name="scps", bufs=4, space="PSUM") as scps,
            tc.tile_pool(name="avps", bufs=2, space="PSUM") as avps,
            tc.tile_pool(name="avsb", bufs=3) as avsb,
        ):
            for h in range(HPC):
                hp = h * 64  # partition offset of this head in vq/vk
                lo = h * (HD + 1)  # free offset of this head (+ones) in vvT
                for j in range(NSCH):
                    e_t = esb.tile([128, TT, SCH], F32, tag="e")
                    for t in range(TT):
                        ps_s = scps.tile([128, SCH], F32, tag="sc_ps")
                        nc.tensor.matmul(
                            ps_s[:],
                            vk_sb[hp:hp + 64, t * 128:(t + 1) * 128],
                            vq_sb[hp:hp + 64, j * SCH:(j + 1) * SCH],
                            start=True, stop=True,
                        )
                        # E = exp(scores / 8)
                        nc.scalar.activation(e_t[:, t, :], ps_s[:], AF.Exp,
                                             scale=1.0 / math.sqrt(HD))
                        # zero the masked diagonal block (t==s)
                        if j * NSCH <= t < (j + 1) * NSCH:
                            col = t * 128 - j * SCH
                            nc.gpsimd.affine_select(
                                out=e_t[:, t, col:col + 128],
                                in_=e_t[:, t, col:col + 128],
                                compare_op=mybir.AluOpType.not_equal,
                                fill=0.0, base=0,
                                pattern=[[-1, 128]], channel_multiplier=1,
                            )
                    ps_h = avps.tile([128, SCH], F32, tag="av_ps")
                    for t in range(TT):
                        nc.tensor.matmul(
                            ps_h[0:HD + 1, :],
                            vvT_sb[:, t, lo:lo + HD + 1],
                            e_t[:, t, :],
                            start=(t == 0), stop=(t == TT - 1),
                        )
                    rec = avsb.tile([1, SCH], F32, tag="av_rec")
                    nc.vector.reciprocal(rec[:], ps_h[HD:HD + 1, :])
                    rb = avsb.tile([HD, SCH], F32, tag="av_rb")
                    nc.gpsimd.partition_broadcast(rb[:], rec[:])
                    ht = avsb.tile([HD, SCH], F32, tag="av_ht")
                    nc.vector.tensor_mul(out=ht[:], in0=ps_h[0:HD, :], in1=rb[:])
                    # scatter the two SL-halves to their destination cores
                    for half in range(SCH // SL):
                        dest = (j * SCH + half * SL) // SL
                        nc.sync.dma_start(
                            send[dest, h * HD:(h + 1) * HD, :],
                            ht[:, half * SL:(half + 1) * SL],
                        )

        # ---------------- Phase C: A2A, Wo, residual + LN1 ----------------
        recv = dram.tile([NCORES, 128, SL], F32, tag="recv")
        nc.gpsimd.collective_compute(
            "AllToAll", mybir.AluOpType.bypass,
            replica_groups=[list(range(NCORES))],
            ins=[send.opt()], outs=[recv.opt()],
        )

        z_sb = attn.tile([128, 2, D], F32)      # LN1 output, [s_tile, s_in, d]
        xT_sb = attn.tile([128, KT, SL], F32)   # z transposed for FFN rhs
        with (
            tc.tile_pool(name="wophase", bufs=1) as woph,
            tc.tile_pool(name="wops", bufs=4, space="PSUM") as wops,
            tc.tile_pool(name="lnsb", bufs=4) as lnsb,
            tc.tile_pool(name="trps2", bufs=2, space="PSUM") as trps2,
        ):
            recvT = woph.tile([128, NCORES, SL], F32)
            nc.sync.dma_start(recvT[:], recv.rearrange("j p s -> p j s"))
            wo_sb = woph.tile([128, KT, D], F32)
            nc.sync.dma_start(wo_sb[:], Wot[:])
            vs_sb = woph.tile([128, 2, D], F32)
            nc.sync.dma_start(vs_sb[:], VsT.ap().rearrange("st p d -> p st d"))

            for st in range(2):  # two tiles of 128 seq positions
                x_sb = lnsb.tile([128, D], F32, tag="x1")
                for nchunk in range(2):
                    ps_o = wops.tile([128, SCH], F32, tag="wo_ps")
                    for k in range(KT):
                        nc.tensor.matmul(
                            ps_o[:],
                            recvT[:, k, st * 128:(st + 1) * 128],
                            wo_sb[:, k, nchunk * SCH:(nchunk + 1) * SCH],
                            start=(k == 0), stop=(k == KT - 1),
                        )
                    nc.vector.tensor_add(
                        out=x_sb[:, nchunk * SCH:(nchunk + 1) * SCH],
                        in0=ps_o[:],
                        in1=vs_sb[:, st, nchunk * SCH:(nchunk + 1) * SCH],
                    )
                _ln(nc, lnsb, x_sb, z_sb[:, st, :], a2_sb, b2n_sb)
                for dt in range(KT):
                    pst = trps2.tile([128, 128], F32, tag="tr2_ps")
                    nc.tensor.transpose(pst[:], z_sb[:, st, dt * 128:(dt + 1) * 128], ident[:])
                    nc.vector.tensor_copy(xT_sb[:, dt, st * 128:(st + 1) * 128], pst[:])

        # ---------------- Phase D: FFN + LN2 + output ----------------
        with (
            tc.tile_pool(name="ffh", bufs=1) as ffhp,
            tc.tile_pool(name="w1s", bufs=3) as w1s,
            tc.tile_pool(name="ffps", bufs=4, space="PSUM") as ffps,
        ):
            ffh_sb = ffhp.tile([128, FFT, SL], F32)
            for m in range(FFT):
                w1_sb = w1s.tile([128, KT, 128], F32, tag="w1")
                nc.sync.dma_start(w1_sb[:], W1t[m])
                ps_f = ffps.tile([128, SL], F32, tag="ff_ps")
                for k in range(KT):
                    nc.tensor.matmul(
                        ps_f[:], w1_sb[:, k, :], xT_sb[:, k, :],
                        start=(k == 0), stop=(k == KT - 1),
                    )
                nc.scalar.activation(ffh_sb[:, m, :], ps_f[:], AF.Relu,
                                     bias=b1_sb[:, m:m + 1], scale=1.0)

            out_sb = attn.tile([128, KT, SL], F32)
            with (
                tc.tile_pool(name="w2s", bufs=2) as w2s,
                tc.tile_pool(name="ff2ps", bufs=2, space="PSUM") as ff2ps,
                tc.tile_pool(name="ln2sb", bufs=4) as ln2sb,
                tc.tile_pool(name="trps3", bufs=2, space="PSUM") as trps3,
            ):
                ffo_sb = ffhp.tile([128, KT, SL], F32)
                for mo in range(KT):
                    w2_sb = w2s.tile([128, FFT, 128], F32, tag="w2")
                    nc.sync.dma_start(w2_sb[:], W2t[mo])
                    ps_g = ff2ps.tile([128, SL], F32, tag="ff2_ps")
                    for k in range(FFT):
                        nc.tensor.matmul(
                            ps_g[:], w2_sb[:, k, :], ffh_sb[:, k, :],
                            start=(k == 0), stop=(k == FFT - 1),
                        )
                    # evacuate + add FFN bias b2 (per-partition)
                    nc.vector.tensor_scalar(
                        out=ffo_sb[:, mo, :], in0=ps_g[:],
                        scalar1=b2_sb[:, mo:mo + 1], scalar2=None,
                        op0=mybir.AluOpType.add,
                    )

                for st in range(2):
                    x2_sb = ln2sb.tile([128, D], F32, tag="x2")
                    for dt in range(KT):
                        pst = trps3.tile([128, 128], F32, tag="tr3_ps")
                        nc.tensor.transpose(pst[:], ffo_sb[:, dt, st * 128:(st + 1) * 128], ident[:])
                        nc.vector.tensor_add(
                            out=x2_sb[:, dt * 128:(dt + 1) * 128],
                            in0=pst[:],
                            in1=z_sb[:, st, dt * 128:(dt + 1) * 128],
                        )
                    z2_sb = ln2sb.tile([128, D], F32, tag="z2")
                    _ln(nc, ln2sb, x2_sb, z2_sb, a2_sb, b2n_sb)
                    for dt in range(KT):
                        pst = trps3.tile([128, 128], F32, tag="tr3_ps")
                        nc.tensor.transpose(pst[:], z2_sb[:, dt * 128:(dt + 1) * 128], ident[:])
                        nc.vector.tensor_copy(out_sb[:, dt, st * 128:(st + 1) * 128], pst[:])

            nc.sync.dma_start(out.ap().rearrange("(dt di) s -> di dt s", di=128), out_sb[:])


_NC_CACHE = {}


def _get_nc(reps: int = 1):
    if reps not in _NC_CACHE:
        _NC_CACHE[reps] = build(reps)
    return _NC_CACHE[reps]


def prep_inputs(Q, K, V, wq, wk, wv, Wo, W1, b1, W2, b2, a_2, b_2):
    """Host-side sharding/layout prep. Returns per-core input maps."""
    f32 = np.float32
    Q = np.asarray(Q, f32); K = np.asarray(K, f32); V = np.asarray(V, f32)
    Qt = np.ascontiguousarray(Q.reshape(KT, 128, S))
    Kt = np.ascontiguousarray(K.reshape(KT, 128, S))
    Vt = np.ascontiguousarray(V.reshape(KT, 128, S))
    Wot = np.ascontiguousarray(np.asarray(Wo, f32).reshape(KT, 128, D).transpose(1, 0, 2))
    W1t = np.ascontiguousarray(np.asarray(W1, f32).reshape(FFT, 128, KT, 128).transpose(0, 3, 2, 1))
    W2t = np.ascontiguousarray(np.asarray(W2, f32).reshape(KT, 128, FFT, 128).transpose(0, 3, 2, 1))
    b1t = np.ascontiguousarray(np.asarray(b1, f32).reshape(FFT, 128).T)
    b2t = np.ascontiguousarray(np.asarray(b2, f32).reshape(KT, 128).T)
    a2b = np.ascontiguousarray(np.broadcast_to(np.asarray(a_2, f32), (128, D)))
    b2nb = np.ascontiguousarray(np.broadcast_to(np.asarray(b_2, f32), (128, D)))

    wq = np.asarray(wq, f32); wk = np.asarray(wk, f32); wv = np.asarray(wv, f32)
    in_maps = []
    for c in range(NCORES):
        # per-core head weights: [2*HD, D] -> lhsT layout [ki, kt, m=128]
        def _wT(w):
            wc = w[c * HPC:(c + 1) * HPC].reshape(128, D)  # [m, k]
            return np.ascontiguousarray(wc.reshape(128, KT, 128).transpose(2, 1, 0))
        VsT = np.ascontiguousarray(
            V[:, c * SL:(c + 1) * SL].T.reshape(2, 128, D))
        in_maps.append({
            "Qt": Qt, "Kt": Kt, "Vt": Vt,
            "wqT": _wT(wq), "wkT": _wT(wk), "wvT": _wT(wv),
            "Wot": Wot, "W1t": W1t, "W2t": W2t,
            "b1t": b1t, "b2t": b2t, "a2b": a2b, "b2nb": b2nb,
            "VsT": VsT,
        })
    return in_maps


def run(in_maps, reps: int = 1):
    nc = _get_nc(reps)
    return run_bass_kernel_spmd(nc, in_maps, list(range(NCORES)))


def kernel(Q, K, V, wq, wk, wv, Wo, W1, b1, W2, b2, a_2, b_2):
    in_maps = prep_inputs(Q, K, V, wq, wk, wv, Wo, W1, b1, W2, b2, a_2, b_2)
    res = run(in_maps, reps=1).results
    return np.concatenate([res[c]["out0"] for c in range(NCORES)], axis=1)
